# revision 58
# baseline (speedup 1.0000x reference)
"""Trainium2 Bass kernel for nn_KANModel (KAN recommender).

Math: with a shared uniform grid (G=5, k=3), the cubic B-spline bases on the
extended uniform knots are shifted cardinal splines, so each KAN layer is
    y = sb*silu(x) + sum_n w_n * relu(u - n)^3,  u = (x - t0)/h,
with n ranging over G+2k+1 = 12 shifts; the truncated-power weights cancel
exactly outside the knot span, so the full block set is exact everywhere.

Layer 0: the embedding tables bound x to [x_min, x_max] exactly, and
u0_min > 4 here, so blocks n <= floor(u0_min) are *always active* and fold
into a single centered cubic polynomial (monomial terms p, p^2, p^3 plus a
constant that joins the bias); only the top blocks stay as relu-cubes.
All relu work happens in x-units (relu(x - tau_n)) with the 1/h^3 scales
folded into the host-side weights, and relu(s)^3 = square(relu(s)) * relu(s).

Orientation: layer-0 output is accumulated TRANSPOSED and with the 64
features duplicated across both partition halves (hT2[o, b], o mod 64) —
matmul cost only depends on the output free size, so the duplication is
free, and it lets layer-1 run feature-major with per-partition-scalar
(kappa-vector) relu blocks, two spline shifts per [128, 128] tile.
The pair-0 kappa vector is folded into the layer-0 bias column so hT2
leaves PSUM already shifted; pair kappas then step by a plain float
2*h1, keeping every layer-1 relu block a cheap SBUF float-scalar op.
Layer-1's 12 blocks become 6 paired {r, q=r*r, z=q*r} triples spread
across DVE/Pool/Act (GPSIMD cannot read PSUM; Act ops carry a large
access shadow, so DVE-from-SBUF does most of the work), with small
accumulating matmuls into y[1, 128], one sigmoid, and a
single-descriptor output DMA.

Data-parallel over batch: 1024 rows -> 8 cores x 128.
"""

import numpy as np

B_FULL = 1024
NCORES = 8
BS = B_FULL // NCORES          # batch shard per core
D = 64                         # embedding dim
IN0, OUT0 = 2 * D, 64          # KAN layer 0
IN1 = 64                       # KAN layer 1 (out_dim 1)
G, KORD = 5, 3
NC_BASIS = G + KORD            # 8 spline bases per edge
NZ = G + 2 * KORD + 1          # 12 possible relu-cube shifts
NU, NI = 100000, 50000

_BUILD_CACHE = {}
TRACE = False
LAST_RESULTS = None

_A5 = np.array([1.0, -4.0, 6.0, -4.0, 1.0], dtype=np.float64) / 6.0


def _m3(s):
    """Cardinal cubic B-spline, exact (clamped) evaluation, float64."""
    s = np.minimum(s, 4.0)
    out = np.zeros_like(s)
    for m in range(4):
        r = np.maximum(s - m, 0.0)
        out += _A5[m] * r * r * r
    return out


def _truncated_power_weights(coef_e, nz):
    """coef_e: (..., NC_BASIS) effective spline coefs -> (nz, ...) relu-cube
    weights w_n (u-units)."""
    out_shape = (nz,) + coef_e.shape[:-1]
    wz = np.zeros(out_shape, dtype=np.float64)
    for n in range(nz):
        for m in range(5):
            c = n - m
            if 0 <= c < NC_BASIS:
                wz[n] += _A5[m] * coef_e[..., c]
    return wz


def _fold_host_weights(grid0, coef0, sb0, ssp0, bias0, grid1, coef1, sb1, ssp1,
                       bias1, x_min, x_max):
    """O(params) host-side prep (f64): folded weights + layouts."""
    h0 = float(grid0[0, -1] - grid0[0, 0]) / G
    t0_0 = float(grid0[0, 0]) - KORD * h0
    h1 = float(grid1[0, -1] - grid1[0, 0]) / G
    t0_1 = float(grid1[0, 0]) - KORD * h1

    # ---- layer 0: block split from exact table extrema ----
    u0_min = (x_min - t0_0) / h0
    u0_max = (x_max - t0_0) / h0
    P0 = [n for n in range(NZ) if n <= u0_min]              # always active
    R0 = [n for n in range(NZ) if u0_min < n < u0_max + 1e-6]

    c0e = (ssp0[:, None].astype(np.float64) * coef0.astype(np.float64)).reshape(
        OUT0, IN0, NC_BASIS
    )  # (o, f, c)
    wz0 = _truncated_power_weights(c0e, NZ)                  # (n, o, f)
    sb0e = sb0.reshape(OUT0, IN0).astype(np.float64)         # (o, f)

    # poly fold: sum_{n in P0} w_n ((x - tau_n)/h0)^3, tau_n = t0_0 + n h0,
    # centered at xc: p = x - xc, delta_n = xc - tau_n
    xc = 0.5 * (x_min + x_max)
    inv_h0_3 = 1.0 / h0**3
    a = {k: np.zeros((OUT0, IN0)) for k in (0, 1, 2, 3)}
    for n in P0:
        dn = xc - (t0_0 + n * h0)
        w = wz0[n] * inv_h0_3
        a[3] += w
        a[2] += 3.0 * dn * w
        a[1] += 3.0 * dn * dn * w
        a[0] += dn**3 * w

    # term order: p, p2, p3, silu, then relu-cube blocks in R0 (x-units)
    # (matches the device-side matmul emission / readiness order)
    terms = [a[1], a[2], a[3], sb0e]
    taus0 = []
    for n in R0:
        terms.append(wz0[n] * inv_h0_3)
        taus0.append(t0_0 + n * h0)
    NT0 = len(terms)
    # lhsT layout [f, o'] with o' = o duplicated across both halves
    W0 = np.zeros((IN0, NT0 * 128), dtype=np.float64)
    for t, w in enumerate(terms):
        W0[:, t * 128 : t * 128 + 64] = w.T
        W0[:, t * 128 + 64 : (t + 1) * 128] = w.T
    W0 = np.ascontiguousarray(W0.astype(np.float32))

    b0c = bias0.astype(np.float64) + a[0].sum(axis=1)        # const per o

    # ---- layer 1: relu-cube blocks only (h-units), paired across halves ----
    # rigorous h range bound (grid + Lipschitz pad) for right-side trim
    NGRID = 2049
    xg = np.linspace(x_min, x_max, NGRID)
    dx = (x_max - x_min) / (NGRID - 1) if x_max > x_min else 0.0
    ug = (xg - t0_0) / h0
    basis = np.stack([_m3(ug - c) for c in range(NC_BASIS)], axis=1)
    silug = xg / (1.0 + np.exp(-xg))
    phi = sb0e[:, :, None] * silug[None, None, :] + np.einsum(
        "ofc,gc->ofg", c0e, basis
    )
    lip = np.abs(sb0e) * 1.1 + np.abs(c0e).sum(axis=2) * (0.75 / h0)
    pad = lip * dx
    h_max = float((bias0.astype(np.float64) + (phi.max(axis=2) + pad).sum(axis=1)).max())
    u1_max = (h_max - t0_1) / h1
    R1 = [n for n in range(NZ) if n < u1_max + 1e-3]

    c1e = ssp1[:, None].astype(np.float64) * coef1.astype(np.float64)  # (f, c)
    wz1 = _truncated_power_weights(c1e, NZ)                  # (n, f)
    inv_h1_3 = 1.0 / h1**3
    NP1 = (len(R1) + 1) // 2
    # wk1 columns: [silu] + NP1 pair-weight cols + [kappa0 vector] + bias1.
    # The pair-0 kappa vector is folded into the layer-0 bias column, so
    # hT2 comes out of PSUM already shifted (= g); true h = hT2 + kappa0.
    wk1 = np.zeros((128, 1 + NP1), dtype=np.float64)
    wk1[0:64, 0] = 0.5 * sb1.astype(np.float64)
    wk1[64:128, 0] = 0.5 * sb1.astype(np.float64)
    kap0 = np.zeros((128, 1), dtype=np.float64)
    kap0[0:64, 0] = t0_1 + R1[0] * h1
    kap0[64:128, 0] = t0_1 + (R1[1] if len(R1) > 1 else R1[0]) * h1
    for j in range(NP1):
        n_top = R1[2 * j]
        wk1[0:64, 1 + j] = wz1[n_top] * inv_h1_3
        if 2 * j + 1 < len(R1):
            n_bot = R1[2 * j + 1]
            wk1[64:128, 1 + j] = wz1[n_bot] * inv_h1_3
    b1col = np.full((128, 1), float(bias1[0]), dtype=np.float64)
    # per-pair Act-Square bias columns: -(j * 2*h1), constant per column
    qb = np.broadcast_to(
        -(np.arange(NP1, dtype=np.float64) * 2.0 * h1)[None, :], (128, NP1)
    )
    wkcat = np.ascontiguousarray(
        np.concatenate([wk1, kap0, b1col, qb], axis=1).astype(np.float32)
    )  # [128, 2*NP1 + 3]

    b0dup = np.ascontiguousarray(
        (np.concatenate([b0c, b0c]) - kap0[:, 0]).reshape(1, 128).astype(np.float32)
    )

    # g-chain requires the retained layer-1 blocks to be consecutive shifts
    assert R1 == list(range(R1[0], R1[0] + len(R1))), R1
    consts = (float(xc), tuple(taus0), NT0, NP1, float(bias1[0]), float(h1))
    return consts, dict(W0=W0, b0dup=b0dup, wkcat=wkcat)


def _build_program(consts):
    import concourse.bass as bass
    import concourse.bacc as bacc
    import concourse.mybir as mybir
    from concourse.tile import TileContext
    from concourse.masks import make_identity

    xc, taus0, NT0, NP1, bias1, h1c = consts
    NR0 = len(taus0)
    f32 = mybir.dt.float32
    i32 = mybir.dt.int32
    A = mybir.AluOpType
    AF = mybir.ActivationFunctionType

    nc = bacc.Bacc("TRN2")
    d_idx = nc.dram_tensor("idx", [BS, 2], i32, kind="ExternalInput")
    d_eu = nc.dram_tensor("emb_user", [NU, D], f32, kind="ExternalInput")
    d_ei = nc.dram_tensor("emb_item", [NI, D], f32, kind="ExternalInput")
    d_w0 = nc.dram_tensor("W0", [IN0, NT0 * 128], f32, kind="ExternalInput")
    d_b0 = nc.dram_tensor("b0dup", [1, 128], f32, kind="ExternalInput")
    WKW = 2 * NP1 + 3
    d_wk = nc.dram_tensor("wkcat", [128, WKW], f32, kind="ExternalInput")
    d_out = nc.dram_tensor("out", [1, BS], f32, kind="ExternalOutput")

    with TileContext(nc) as tc:
        with (
            tc.tile_pool(name="sb", bufs=1) as P,
            tc.tile_pool(name="ps", bufs=1, space="PSUM") as PS,
        ):
            # --- input DMAs; idx first so the gathers start ASAP ---
            idx = P.tile([BS, 2], i32, tag="idx")
            nc.sync.dma_start(out=idx[:], in_=d_idx[:])
            b0 = P.tile([1, 128], f32, tag="b0")
            nc.sync.dma_start(out=b0[:1, :], in_=d_b0[:])
            w0 = P.tile([IN0, NT0 * 128], f32, tag="w0")
            nc.sync.dma_start(out=w0[:], in_=d_w0[:])
            wk = P.tile([128, WKW], f32, tag="wk")
            nc.sync.dma_start(out=wk[:], in_=d_wk[:])

            # --- early Pool-side setup, then the gathers ---
            ones = P.tile([1, 128], f32, tag="ones")
            nc.gpsimd.memset(ones[:1, :], 1.0)
            ident = P.tile([128, 128], f32, tag="ident")
            make_identity(nc, ident[:])

            xbm = P.tile([BS, 2 * D], f32, tag="xbm")
            nc.gpsimd.indirect_dma_start(
                out=xbm[:, 0:D], out_offset=None, in_=d_eu[:],
                in_offset=bass.IndirectOffsetOnAxis(ap=idx[:, 0:1], axis=0),
            )
            nc.gpsimd.indirect_dma_start(
                out=xbm[:, D : 2 * D], out_offset=None, in_=d_ei[:],
                in_offset=bass.IndirectOffsetOnAxis(ap=idx[:, 1:2], axis=0),
            )

            # --- PE: clock warmup, bias seed, transpose ---
            warm = PS.tile([128, 8], f32, tag="warm")
            nc.tensor.matmul(out=warm[:], lhsT=ones[:1, :], rhs=ones[:1, 0:8],
                             start=True, stop=True)
            hT2 = PS.tile([128, BS], f32, tag="hT2")
            nc.tensor.matmul(out=hT2[:], lhsT=b0[:1, :], rhs=ones[:1, :],
                             start=True, stop=False)
            xT = PS.tile([128, BS], f32, tag="xT")
            nc.tensor.matmul(out=xT[:], lhsT=xbm[:], rhs=ident[:],
                             is_transpose=True, start=True, stop=True)

            # --- layer 0 elementwise terms (feature-major [128, BS]) ---
            # One PSUM->SBUF hop (tp); everything downstream runs on SBUF
            # where DVE ops are ~2x cheaper. GPSIMD cannot touch PSUM.
            # Matmuls are emitted in term-readiness order (PE dispatches
            # in order): monomials first, relu-cube blocks last.
            tp = P.tile([128, BS], f32, tag="tp")
            nc.vector.tensor_scalar(tp[:], xT[:], float(xc), None, A.subtract)
            sg0 = P.tile([128, BS], f32, tag="sg0")
            nc.scalar.activation(sg0[:], xT[:], AF.Sigmoid)

            tp2 = P.tile([128, BS], f32, tag="tp2")
            nc.vector.tensor_tensor(out=tp2[:], in0=tp[:], in1=tp[:], op=A.mult)
            tp3 = P.tile([128, BS], f32, tag="tp3")
            nc.vector.tensor_tensor(out=tp3[:], in0=tp2[:], in1=tp[:], op=A.mult)

            rsl = P.tile([128, NR0 * BS], f32, tag="rsl")
            qsl = P.tile([128, NR0 * BS], f32, tag="qsl")
            zsl = P.tile([128, NR0 * BS], f32, tag="zsl")
            # block 0 rides a Pool chain; block 1 goes DVE(r) -> Act(q) ->
            # DVE(z) so the two chains run in parallel lanes.
            for k, tau in enumerate(taus0):
                sl = slice(k * BS, (k + 1) * BS)
                engr = nc.gpsimd if k % 2 == 0 else nc.vector
                engr.tensor_scalar(rsl[:, sl], tp[:], float(tau - xc), 0.0,
                                   A.subtract, A.max)
                if k % 2 == 0:
                    nc.gpsimd.tensor_tensor(out=qsl[:, sl], in0=rsl[:, sl],
                                            in1=rsl[:, sl], op=A.mult)
                    nc.gpsimd.tensor_tensor(out=zsl[:, sl], in0=qsl[:, sl],
                                            in1=rsl[:, sl], op=A.mult)
                else:
                    nc.scalar.activation(qsl[:, sl], rsl[:, sl], AF.Square)
                    nc.vector.tensor_tensor(out=zsl[:, sl], in0=qsl[:, sl],
                                            in1=rsl[:, sl], op=A.mult)

            tsilu = P.tile([128, BS], f32, tag="tsilu")
            nc.vector.scalar_tensor_tensor(tsilu[:], tp[:], float(xc), sg0[:],
                                           A.add, A.mult)

            # --- layer 0 matmuls (readiness order; W0 cols follow suit) ---
            l0_rhs = [tp[:], tp2[:], tp3[:], tsilu[:]] + [
                zsl[:, k * BS : (k + 1) * BS] for k in range(NR0)
            ]
            for t, rhs in enumerate(l0_rhs):
                nc.tensor.matmul(
                    out=hT2[:], lhsT=w0[:, t * 128 : (t + 1) * 128], rhs=rhs,
                    start=False, stop=(t == len(l0_rhs) - 1),
                )

            # --- layer 1 (feature-major, features duplicated across halves) ---
            # hT2 comes out of PSUM pre-shifted by the pair-0 kappa vector
            # (folded into the bias column), so g is a plain copy and every
            # relu block is a float-scalar op on SBUF data. Pair kappas step
            # uniformly by 2*h1. Lanes: triples 0..NP1-4 on DVE, the next
            # two on Pool, the last r/z on DVE with its square on Act.
            kstep = 2.0 * float(h1c)
            kap0c = wk[:, NP1 + 1 : NP1 + 2]
            g = P.tile([128, BS], f32, tag="g")
            nc.vector.tensor_scalar(g[:], hT2[:], 0.0, None, A.add)
            tS = P.tile([128, BS], f32, tag="tS")
            nc.scalar.activation(tS[:], hT2[:], AF.Sigmoid, bias=kap0c)
            tsH = P.tile([128, BS], f32, tag="tsH")
            nc.vector.scalar_tensor_tensor(tsH[:], g[:], kap0c, tS[:],
                                           A.add, A.mult)

            y = PS.tile([1, BS], f32, tag="y")
            nc.tensor.matmul(out=y[:1, :], lhsT=wk[:, 0:1], rhs=tsH[:],
                             start=True, stop=False)

            rsb = P.tile([128, NP1 * BS], f32, tag="rsb")
            qsb = P.tile([128, NP1 * BS], f32, tag="qsb")
            zsb = P.tile([128, NP1 * BS], f32, tag="zsb")
            # lane maps: r/z engine per pair, q source per pair. Act q's run
            # directly from PSUM (Square(hT2' + qb_j)), independent of r_j,
            # so pairs 0/1 have depth-1 chains and DVE sheds two ops.
            if NP1 == 6:
                rz_pool = {3, 4}
                q_act = {2, 4, 5}
                q_pool = {3}
                order1 = [5, 0, 3, 1, 4, 2]
                mm_order = [5, 0, 1, 3, 4, 2]
            else:
                rz_pool = {j for j in range(NP1) if j >= max(NP1 - 3, 1)
                           and j < NP1 - 1}
                q_act = set()
                q_pool = rz_pool
                order1 = list(range(NP1))
                mm_order = list(range(NP1))
            for j in order1:
                sl = slice(j * BS, (j + 1) * BS)
                engr = nc.gpsimd if j in rz_pool else nc.vector
                engr.tensor_scalar(rsb[:, sl], g[:], j * kstep, 0.0,
                                   A.subtract, A.max)
                if j in q_act:
                    if j == 0:
                        nc.scalar.activation(qsb[:, sl], hT2[:], AF.Square)
                    else:
                        nc.scalar.activation(qsb[:, sl], hT2[:], AF.Square,
                                             bias=wk[:, NP1 + 3 + j :
                                                     NP1 + 4 + j])
                else:
                    engq = nc.gpsimd if j in q_pool else nc.vector
                    engq.tensor_tensor(out=qsb[:, sl], in0=rsb[:, sl],
                                       in1=rsb[:, sl], op=A.mult)
                engz = nc.gpsimd if j in rz_pool else nc.vector
                engz.tensor_tensor(out=zsb[:, sl], in0=qsb[:, sl],
                                   in1=rsb[:, sl], op=A.mult)
            for i, j in enumerate(mm_order):
                nc.tensor.matmul(out=y[:1, :], lhsT=wk[:, 1 + j : 2 + j],
                                 rhs=zsb[:, j * BS : (j + 1) * BS],
                                 start=False, stop=(i == NP1 - 1))

            osb = P.tile([1, BS], f32, tag="osb")
            nc.scalar.activation(osb[0:1, :], y[:1, :], AF.Sigmoid,
                                 bias=wk[0:1, NP1 + 2 : NP1 + 3])
            nc.sync.dma_start(out=d_out[:], in_=osb[:1, :])

    nc.compile()
    return nc


def kernel(
    user_indices, item_indices, grid_update_num, stop_grid_update_step,
    emb_user, emb_item,
    grid0, coef0, sb0, ssp0, bias0,
    grid1, coef1, sb1, ssp1, bias1,
):
    global LAST_RESULTS
    from concourse.bass_utils import run_bass_kernel_spmd

    uidx = np.asarray(user_indices).astype(np.int32).reshape(B_FULL, 1)
    iidx = np.asarray(item_indices).astype(np.int32).reshape(B_FULL, 1)
    eu = np.ascontiguousarray(np.asarray(emb_user, dtype=np.float32))
    ei = np.ascontiguousarray(np.asarray(emb_item, dtype=np.float32))
    x_min = float(min(eu.min(), ei.min()))
    x_max = float(max(eu.max(), ei.max()))

    consts, w = _fold_host_weights(
        np.asarray(grid0, dtype=np.float32), np.asarray(coef0, dtype=np.float32),
        np.asarray(sb0, dtype=np.float32), np.asarray(ssp0, dtype=np.float32),
        np.asarray(bias0, dtype=np.float32), np.asarray(grid1, dtype=np.float32),
        np.asarray(coef1, dtype=np.float32), np.asarray(sb1, dtype=np.float32),
        np.asarray(ssp1, dtype=np.float32), np.asarray(bias1, dtype=np.float32),
        x_min, x_max,
    )

    if consts not in _BUILD_CACHE:
        _BUILD_CACHE[consts] = _build_program(consts)
    nc = _BUILD_CACHE[consts]

    in_maps = []
    for c in range(NCORES):
        sl = slice(c * BS, (c + 1) * BS)
        in_maps.append(
            {
                "idx": np.ascontiguousarray(
                    np.concatenate([uidx[sl], iidx[sl]], axis=1)),
                "emb_user": eu,
                "emb_item": ei,
                "W0": w["W0"],
                "b0dup": w["b0dup"],
                "wkcat": w["wkcat"],
            }
        )

    res = run_bass_kernel_spmd(nc, in_maps, core_ids=list(range(NCORES)),
                               trace=TRACE)
    LAST_RESULTS = res
    return np.concatenate(
        [r["out"].reshape(BS, 1) for r in res.results], axis=0
    )
